# revision 1
# baseline (speedup 1.0000x reference)
"""Trainium2 Bass kernel for nn_RandomLayer.

Computes out[b, o] = sum_{i,j} features[b, i] * softmax[b, j] * R[i*C + j, o]
  with B=512, D=2048, C=100, O=1024  (R is [204800, 1024] fp32, ~839 MB).

Strategy:
  * Shard the O=1024 output columns across 8 NeuronCores (128 each). No
    communication needed; host concatenates the per-core outputs.
  * Per core, restructure as: for each class j: P_j = F @ R[:, j, :]
    (a [512,2048]x[2048,128] matmul), then out += diag(s[:, j]) @ P_j.
  * Matmul layout keeps the batch on PSUM partitions so the s[b, j] scale is
    a per-partition scalar -> one fused scalar_tensor_tensor (FMA) per drain.
  * j's are processed 4 at a time so the moving operand is [128, 512]
    (full PSUM bank). Per core: 25 j-groups x 4 batch-blocks x 16 K-tiles
    = 1600 matmuls of [K=128, M=128] x [K=128, N=512] in bf16.
  * R is cast to bf16 and pre-permuted on the host so each j-group's moving
    operand is one contiguous 2 MB DMA (16 KB per partition).
"""

import numpy as np
import ml_dtypes

import concourse.bass as bass
import concourse.mybir as mybir
import concourse.tile as tile
from concourse import bacc
from concourse.bass_utils import run_bass_kernel_spmd

B, D, C, O = 512, 2048, 100, 1024
NCORES = 8
OS = O // NCORES  # 128 output cols per core
T = D // 128      # 16 contraction tiles
JG = C // 4       # 25 j-groups (4 classes each)
BB = B // 128     # 4 batch blocks

_BF16 = ml_dtypes.bfloat16

_CACHE = {}


def _build_nc():
    """Build + compile the per-core Bass program (same program on all cores)."""
    dt = mybir.dt
    nc = bacc.Bacc("TRN2", target_bir_lowering=False, debug=False)

    ft_d = nc.dram_tensor("ft", [128, T * BB * 128], dt.bfloat16, kind="ExternalInput")
    sc_d = nc.dram_tensor("sc", [128, BB * C], dt.float32, kind="ExternalInput")
    rm_d = nc.dram_tensor("rm", [JG, 128, T * 512], dt.bfloat16, kind="ExternalInput")
    out_d = nc.dram_tensor("out", [BB, 128, OS], dt.float32, kind="ExternalOutput")

    with tile.TileContext(nc) as tc:
        with (
            tc.tile_pool(name="const", bufs=1) as constp,
            tc.tile_pool(name="rmp", bufs=3) as rmp,
            tc.tile_pool(name="psp", bufs=6, space=bass.MemorySpace.PSUM) as psp,
        ):
            ft = constp.tile([128, T * BB * 128], dt.bfloat16)
            nc.sync.dma_start(ft[:], ft_d[:])
            sc = constp.tile([128, BB * C], dt.float32)
            nc.sync.dma_start(sc[:], sc_d[:])
            acc = constp.tile([128, BB * OS], dt.float32)
            nc.vector.memset(acc[:], 0.0)

            for jg in range(JG):
                rm = rmp.tile([128, T * 512], dt.bfloat16)
                nc.sync.dma_start(rm[:], rm_d[jg])
                for m in range(BB):
                    ps = psp.tile([128, 512], dt.float32)
                    for t in range(T):
                        nc.tensor.matmul(
                            ps[:],
                            ft[:, (t * BB + m) * 128 : (t * BB + m + 1) * 128],
                            rm[:, t * 512 : (t + 1) * 512],
                            start=(t == 0),
                            stop=(t == T - 1),
                        )
                    for jj in range(4):
                        j = jg * 4 + jj
                        nc.vector.scalar_tensor_tensor(
                            out=acc[:, m * OS : (m + 1) * OS],
                            in0=ps[:, jj * OS : (jj + 1) * OS],
                            scalar=sc[:, m * C + j : m * C + j + 1],
                            in1=acc[:, m * OS : (m + 1) * OS],
                            op0=mybir.AluOpType.mult,
                            op1=mybir.AluOpType.add,
                        )

            for m in range(BB):
                nc.sync.dma_start(out_d[m], acc[:, m * OS : (m + 1) * OS])

    nc.compile()
    return nc


def _prep_inputs(features, softmax_output, random_matrix):
    """Host-side cast + permute into the DMA-friendly layouts."""
    # ft[p, (t*BB + m)*128 + bl] = F[m*128 + bl, t*128 + p]   (bf16)
    ft = np.ascontiguousarray(
        features.astype(_BF16).reshape(BB, 128, T, 128).transpose(3, 2, 0, 1)
    ).reshape(128, T * BB * 128)

    # sc[p, m*C + j] = s[m*128 + p, j]   (fp32)
    sc = np.ascontiguousarray(
        softmax_output.astype(np.float32).reshape(BB, 128, C).transpose(1, 0, 2)
    ).reshape(128, BB * C)

    # rm_c[jg, p, t*512 + jj*128 + o] = R[(t*128+p)*C + (jg*4+jj), c*OS + o]
    r6 = random_matrix.astype(_BF16).reshape(T, 128, JG, 4, NCORES, OS)
    rm_all = np.ascontiguousarray(r6.transpose(4, 2, 1, 0, 3, 5))
    rms = [rm_all[c].reshape(JG, 128, T * 512) for c in range(NCORES)]

    return [{"ft": ft, "sc": sc, "rm": rms[c]} for c in range(NCORES)]


def _run(inputs, trace=False, **kwargs):
    if "nc" not in _CACHE:
        _CACHE["nc"] = _build_nc()
    nc = _CACHE["nc"]
    in_maps = _prep_inputs(
        inputs["features"], inputs["softmax_output"], inputs["random_matrix"]
    )
    res = run_bass_kernel_spmd(nc, in_maps, list(range(NCORES)), trace=trace, **kwargs)
    out = np.concatenate(
        [res.results[c]["out"].reshape(B, OS) for c in range(NCORES)], axis=1
    ).astype(np.float32)
    return out, res


def kernel(features, softmax_output, random_matrix):
    out, _ = _run(
        {
            "features": features,
            "softmax_output": softmax_output,
            "random_matrix": random_matrix,
        }
    )
    return out


# revision 3
# speedup vs baseline: 315690.1325x; 315690.1325x over previous
"""Trainium2 Bass kernel for nn_RandomLayer.

Computes out[b, o] = sum_{i,j} features[b, i] * softmax[b, j] * R[i*C + j, o]
  with B=512, D=2048, C=100, O=1024  (R is [204800, 1024] fp32, ~839 MB).

Strategy:
  * Shard the O=1024 output columns across 8 NeuronCores (128 each). No
    communication needed; host concatenates the per-core outputs.
  * Per core, restructure as: for each class j: P_j = F @ R[:, j, :]
    (a [512,2048]x[2048,128] matmul), then out += diag(s[:, j]) @ P_j.
  * Matmul layout keeps the batch on PSUM partitions so the s[b, j] scale is
    a per-partition scalar -> one fused scalar_tensor_tensor (FMA) per drain.
  * j's are processed 4 at a time so the moving operand is [128, 512]
    (full PSUM bank). Per core: 25 j-groups x 4 batch-blocks x 16 K-tiles
    = 1600 matmuls of [K=128, M=128] x [K=128, N=512] in bf16.
  * R is cast to bf16 and pre-permuted on the host so each j-group's moving
    operand is one contiguous 2 MB DMA (16 KB per partition).
"""

import numpy as np
import ml_dtypes

import jax
from jax.experimental.shard_map import shard_map
from jax.sharding import Mesh, PartitionSpec

import concourse.bass as bass
import concourse.mybir as mybir
import concourse.tile as tile
from concourse import bacc
from concourse import bass2jax as b2j

B, D, C, O = 512, 2048, 100, 1024
NCORES = 8
OS = O // NCORES  # 128 output cols per core
T = D // 128      # 16 contraction tiles
JG = C // 4       # 25 j-groups (4 classes each)
BB = B // 128     # 4 batch blocks

_BF16 = ml_dtypes.bfloat16

_CACHE = {}


def _build_nc():
    """Build + compile the per-core Bass program (same program on all cores)."""
    dt = mybir.dt
    nc = bacc.Bacc("TRN2", target_bir_lowering=False, debug=False)

    ft_d = nc.dram_tensor("ft", [128, T * BB * 128], dt.bfloat16, kind="ExternalInput")
    sc_d = nc.dram_tensor("sc", [128, BB * C], dt.float32, kind="ExternalInput")
    rm_d = nc.dram_tensor("rm", [JG, 128, T * 512], dt.bfloat16, kind="ExternalInput")
    out_d = nc.dram_tensor("out", [BB, 128, OS], dt.float32, kind="ExternalOutput")

    with tile.TileContext(nc) as tc:
        with (
            tc.tile_pool(name="const", bufs=1) as constp,
            tc.tile_pool(name="rmp", bufs=3) as rmp,
            tc.tile_pool(name="psp", bufs=6, space=bass.MemorySpace.PSUM) as psp,
        ):
            ft = constp.tile([128, T * BB * 128], dt.bfloat16)
            nc.sync.dma_start(ft[:], ft_d[:])
            sc = constp.tile([128, BB * C], dt.float32)
            nc.sync.dma_start(sc[:], sc_d[:])
            acc = constp.tile([128, BB * OS], dt.float32)
            nc.vector.memset(acc[:], 0.0)

            for jg in range(JG):
                rm = rmp.tile([128, T * 512], dt.bfloat16)
                nc.sync.dma_start(rm[:], rm_d[jg])
                for m in range(BB):
                    ps = psp.tile([128, 512], dt.float32)
                    for t in range(T):
                        nc.tensor.matmul(
                            ps[:],
                            ft[:, (t * BB + m) * 128 : (t * BB + m + 1) * 128],
                            rm[:, t * 512 : (t + 1) * 512],
                            start=(t == 0),
                            stop=(t == T - 1),
                        )
                    for jj in range(4):
                        j = jg * 4 + jj
                        nc.vector.scalar_tensor_tensor(
                            out=acc[:, m * OS : (m + 1) * OS],
                            in0=ps[:, jj * OS : (jj + 1) * OS],
                            scalar=sc[:, m * C + j : m * C + j + 1],
                            in1=acc[:, m * OS : (m + 1) * OS],
                            op0=mybir.AluOpType.mult,
                            op1=mybir.AluOpType.add,
                        )

            for m in range(BB):
                nc.sync.dma_start(out_d[m], acc[:, m * OS : (m + 1) * OS])

    nc.compile()
    return nc


def _make_exec(nc, reps=1):
    """Build a cached, jitted SPMD executable for `nc` that runs the NEFF
    `reps` times back to back (outputs chained into the next run's output
    buffers, forcing sequential execution). Mirrors bass2jax.run_bass_via_pjrt
    but reuses one jitted callable across calls."""
    b2j.install_neuronx_cc_hook()

    partition_name = nc.partition_id_tensor.name if nc.partition_id_tensor else None
    in_names, out_names, out_avals, zero_outs = [], [], [], []
    for alloc in nc.m.functions[0].allocations:
        if not isinstance(alloc, mybir.MemoryLocationSet):
            continue
        name = alloc.memorylocations[0].name
        if alloc.kind == "ExternalInput":
            if name != partition_name:
                in_names.append(name)
        elif alloc.kind == "ExternalOutput":
            assert alloc.tensor_shape is not None and alloc.dtype is not None
            out_names.append(name)
            shape = tuple(alloc.tensor_shape)
            dtype = mybir.dt.np(alloc.dtype)
            out_avals.append(jax.core.ShapedArray(shape, dtype))
            zero_outs.append(np.zeros(shape, dtype))
    n_params = len(in_names)
    all_names = tuple(in_names + out_names + ([partition_name] if partition_name else []))

    def _body(*args):
        ins = list(args[:n_params])
        outs = list(args[n_params:])
        pid = [b2j.partition_id_tensor()] if partition_name else []
        # `reps` independent calls on identical parameter operands; BassEffect
        # is an ordered effect, so XLA serializes them (and cannot CSE/DCE).
        for _ in range(reps):
            res = list(
                b2j._bass_exec_p.bind(
                    *(ins + outs + pid),
                    out_avals=tuple(out_avals),
                    in_names=all_names,
                    out_names=tuple(out_names),
                    lowering_input_output_aliases=(),
                    sim_require_finite=False,
                    sim_require_nnan=False,
                    nc=nc,
                )
            )
        return tuple(res)

    devices = jax.devices()[:NCORES]
    mesh = Mesh(np.asarray(devices), ("core",))
    n_args = n_params + len(out_names)
    fn = jax.jit(
        shard_map(
            _body,
            mesh=mesh,
            in_specs=(PartitionSpec("core"),) * n_args,
            out_specs=(PartitionSpec("core"),) * len(out_names),
            check_rep=False,
        )
    )
    return fn, in_names, out_names, zero_outs, mesh


def _get_exec(reps=1):
    key = ("exec", reps)
    if key not in _CACHE:
        if "nc" not in _CACHE:
            _CACHE["nc"] = _build_nc()
        _CACHE[key] = _make_exec(_CACHE["nc"], reps)
    return _CACHE[key]


def _prep_inputs(features, softmax_output, random_matrix):
    """Host-side cast + permute into the DMA-friendly layouts.

    Returns the axis-0-concatenated global arrays {name: array} where each
    core's shard is its [k*n : (k+1)*n] slice along axis 0."""
    # ft[p, (t*BB + m)*128 + bl] = F[m*128 + bl, t*128 + p]   (bf16, replicated)
    ft = np.ascontiguousarray(
        features.astype(_BF16).reshape(BB, 128, T, 128).transpose(3, 2, 0, 1)
    ).reshape(128, T * BB * 128)

    # sc[p, m*C + j] = s[m*128 + p, j]   (fp32, replicated)
    sc = np.ascontiguousarray(
        softmax_output.astype(np.float32).reshape(BB, 128, C).transpose(1, 0, 2)
    ).reshape(128, BB * C)

    # rm_c[jg, p, t*512 + jj*128 + o] = R[(t*128+p)*C + (jg*4+jj), c*OS + o]
    r6 = random_matrix.astype(_BF16).reshape(T, 128, JG, 4, NCORES, OS)
    rm_all = np.ascontiguousarray(r6.transpose(4, 2, 1, 0, 3, 5))  # [c,jg,p,t,jj,o]

    return {
        "ft": np.broadcast_to(ft, (NCORES,) + ft.shape).reshape(NCORES * 128, -1),
        "sc": np.broadcast_to(sc, (NCORES,) + sc.shape).reshape(NCORES * 128, -1),
        "rm": rm_all.reshape(NCORES * JG, 128, T * 512),
    }


def _run(inputs, reps=1, device_args=None):
    """Run the SPMD kernel; returns (full_output, device_out_list)."""
    fn, in_names, out_names, zero_outs, mesh = _get_exec(reps)
    if device_args is None:
        device_args = _put_args(inputs)
    out_arrs = fn(*device_args)
    out = np.asarray(out_arrs[0]).reshape(NCORES, B, OS)
    return np.ascontiguousarray(out.transpose(1, 0, 2)).reshape(B, O), out_arrs


def _put_args(inputs):
    fn, in_names, out_names, zero_outs, mesh = _get_exec(1)
    globals_ = _prep_inputs(
        inputs["features"], inputs["softmax_output"], inputs["random_matrix"]
    )
    args = [globals_[n] for n in in_names]
    args += [
        np.zeros((NCORES * z.shape[0],) + z.shape[1:], z.dtype) for z in zero_outs
    ]
    sharding = jax.sharding.NamedSharding(mesh, PartitionSpec("core"))
    return [jax.device_put(a, sharding) for a in args]


def kernel(features, softmax_output, random_matrix):
    out, _ = _run(
        {
            "features": features,
            "softmax_output": softmax_output,
            "random_matrix": random_matrix,
        }
    )
    return out
